# revision 25
# baseline (speedup 1.0000x reference)
"""GQA attention layer (B=4, S=2048, D=2048, 16 heads / 4 KV heads, RoPE,
causal) on 8 trn2 NeuronCores.

Sharding: TP=4 over KV-head groups x DP=2 over batch. Each core handles 2
batches and one KV group (4 q heads + 1 kv head), computes a partial
(head-group) contribution to out = attn @ wo; host sums the 4 partials per
batch group.

Device layout choices:
  - host pre-casts to bf16 and pre-tiles everything into [128, ...]
    partition-contiguous layouts (pure per-partition memcpy DMAs), and
    permutes wq/wk columns per head to "evens then odds" so RoPE becomes
    rotate-half.
  - DMA triggers are spread across the idle Tensor/Vector/GpSimd sequencer
    queues (each DIRECT2D trigger costs ~0.6us of sequencer time; the Sync
    queue alone serializes them). wk/wv are loaded first and slice 0 runs
    k/v projections before q so the PE can start ~2us in on the small
    weights while wq streams.
  - q,k are produced transposed ([dh, tok]) straight from the projection
    matmuls; v is produced natural ([tok, dh]).
  - RoPE in transposed layout: rot = q * C2 + swap_halves(q) * S2 with
    C2 = [cos;cos], S2 = [-sin;+sin]; the half swap is 2 SBUF->SBUF DMAs.
  - attention: scoresT = kT_tile.T @ qT (k on partitions), exp on ACT (no
    max subtraction -- scores are O(5) here). Causal masking stays on the
    Tensor engine: diagonal tiles get -30000 added via an extra
    identity @ mask matmul (narrowed to the 128-col diagonal block)
    accumulated into the scores PSUM group, and all ops on a diagonal tile
    skip its fully-masked first 128*r columns.
    PV matmuls accumulate in PSUM over k-tiles. The softmax denominator
    ones-matmul is quad-batched for off-diagonal tiles: 4 exp tiles are
    tree-summed on DVE (3 bf16 adds) and a single ones @ esum matmul
    accumulates into l (4x fewer PE cycles); diagonal tiles keep per-tile
    ones-matmuls narrowed by qlo. 1/l via DVE reciprocal_approx_fast.
  - wo outputs for one 128-token row block are collected into a single
    [128, 4, 512] f32 SBUF tile and written with ONE 1MB DMA (contiguous
    8KB per partition line, split across all 16 SDMA engines) triggered on
    the idle Sync sequencer: 32 triggers total instead of 256 on GpSimd,
    which removes the out-DMA completion stalls and the serialized
    trigger tail after the last matmul.
  - wo blocks for iteration (b, qs) are emitted one-per-head during
    iteration (b, qs)+1's attention loop, so the PE FIFO interleaves wo
    matmuls with attention matmuls: exp-latency stalls are filled by wo
    work and psum-copy stalls by attention work. This also keeps the PE
    HAM clock-gate warm (K=8/8) through all of phase C.
"""

import itertools
import math
from contextlib import ExitStack

import ml_dtypes
import numpy as np

import concourse.bass as bass
import concourse.mybir as mybir
import concourse.tile as tile
from concourse import bacc
from concourse.bass_utils import run_bass_kernel_spmd

BF16 = mybir.dt.bfloat16
F32 = mybir.dt.float32

# Full-problem constants (hardcoded per harness contract)
B, S, D = 4, 2048, 2048
NH, NKV, DH = 16, 4, 128
TP, DP = 4, 2
BL = B // DP          # batches per core
T = BL * S            # tokens per core
HL = NH // TP         # q heads per core
QC = HL * DH          # q cols per core
NT128 = S // 128      # 128-token tiles per batch (16)
NSL = S // 512        # 512-token slices per batch (4)
KD = D // 128         # contraction tiles for the projections (16)


def build_nc(sc_bufs=3, oT_bufs=2, wo_bufs=2, exp_bufs=10, xt_bufs=2,
             q_bufs=2, k_bufs=2, v_bufs=2, wo_copy="mix",
             psb_bufs=3, asb_bufs=4, out_bufs=3, l_quad=True):
    nc = bacc.Bacc("TRN2", target_bir_lowering=False, debug=False)

    xt = nc.dram_tensor("xt", [128, T // 512, KD, 512], BF16,
                        kind="ExternalInput").ap()
    wq = nc.dram_tensor("wq", [128, KD, QC], BF16, kind="ExternalInput").ap()
    wk = nc.dram_tensor("wk", [128, KD, DH], BF16, kind="ExternalInput").ap()
    wv = nc.dram_tensor("wv", [128, KD, DH], BF16, kind="ExternalInput").ap()
    wo = nc.dram_tensor("wo", [128, HL, D], BF16, kind="ExternalInput").ap()
    cos2 = nc.dram_tensor("cos2", [128, S], BF16, kind="ExternalInput").ap()
    sin2 = nc.dram_tensor("sin2", [128, S], BF16, kind="ExternalInput").ap()
    msk = nc.dram_tensor("msk", [128, 4, 512], BF16, kind="ExternalInput").ap()
    ident = nc.dram_tensor("ident", [128, 128], BF16, kind="ExternalInput").ap()
    # [T, D] viewed as [T, 4, 512] so a [128, 4, 512] SBUF tile DMAs as one
    # fully-contiguous 1MB transfer per 128-token row block.
    out = nc.dram_tensor("out", [T, 4, 512], F32, kind="ExternalOutput").ap()

    scale = 1.0 / math.sqrt(DH)

    with tile.TileContext(nc) as tc, ExitStack() as ctx:
        persist = ctx.enter_context(tc.tile_pool(name="persist", bufs=1))

        # --- resident weights / tables ---
        wq_sb = persist.tile([128, KD, QC], BF16, tag="wq")
        wk_sb = persist.tile([128, KD, DH], BF16, tag="wk")
        wv_sb = persist.tile([128, KD, DH], BF16, tag="wv")
        wo_sb = persist.tile([128, HL, D], BF16, tag="wo")
        cos_sb = persist.tile([128, S], BF16, tag="cos")
        sin_sb = persist.tile([128, S], BF16, tag="sin")
        msk_sb = persist.tile([128, 4, 512], BF16, tag="msk")
        ident_sb = persist.tile([128, 128], BF16, tag="ident")
        ones_sb = persist.tile([128, 128], BF16, tag="ones")
        nc.vector.memset(ones_sb[:], 1.0)

        # --- resident activations ---
        qT_sb = persist.tile([128, HL, BL, S], BF16, tag="qT")
        kT_sb = persist.tile([128, BL, S], BF16, tag="kT")
        v_sb = persist.tile([128, BL, NT128, DH], BF16, tag="v")

        # ---------------- phase B: projections + RoPE ----------------
        with tc.tile_pool(name="proj_sb", bufs=psb_bufs) as psb, \
             tc.tile_pool(name="proj_ps", bufs=2, space="PSUM") as pps:

            # Pre-allocate the xt tiles for all slices up-front so slice 0's
            # load can be emitted first with fine-grained chunking.
            xt_tiles = [
                psb.tile([128, KD, 512], BF16, tag="xt", name="xt_sl",
                         bufs=xt_bufs)
                for _ in range(T // 512)
            ]

            # Highest priority: wk/wv (small, unblock k/v projections fast),
            # then xt slice 0 and wq in 4-o chunks (512KB: each DMA already
            # splits across all 16 SDMA engines, so fewer/bigger triggers
            # beat fine chunking -- each DIRECT2D trigger costs ~0.6us of
            # sequencer time), then slice 1. RoPE tables go early on sync
            # (cos/sin are needed by slice 0's rope); the phase-C tables
            # ride on gpsimd behind slice 1.
            # Small chunks are slow per-transfer (descriptor-dominated once
            # split 16 ways), so what matters is spreading them round-robin
            # across the three trigger queues so several are in flight at
            # once -- NOT dependency-ordering them on one queue.
            nc.scalar.dma_start(wk_sb[:], wk)
            nc.gpsimd.dma_start(wv_sb[:], wv)
            trig = itertools.cycle([nc.sync, nc.scalar, nc.gpsimd])
            for osl in (slice(0, 2), slice(2, 4), slice(4, 6), slice(6, 8),
                        slice(8, 12), slice(12, 16)):
                next(trig).dma_start(xt_tiles[0][:, osl, :], xt[:, 0, osl, :])
            # rope tables before wq: the first k-rope needs cos/sin ~7us
            # before the first q matmul needs wq
            nc.sync.dma_start(cos_sb[:], cos2)
            nc.sync.dma_start(sin_sb[:], sin2)
            for oc in range(4):
                osl = slice(4 * oc, 4 * oc + 4)
                next(trig).dma_start(wq_sb[:, osl, :], wq[:, osl, :])
            for oc in range(2):
                osl = slice(8 * oc, 8 * oc + 8)
                next(trig).dma_start(xt_tiles[1][:, osl, :], xt[:, 1, osl, :])
            for c in range(HL):
                nc.gpsimd.dma_start(wo_sb[:, c, :], wo[:, c, :])
            nc.gpsimd.dma_start(msk_sb[:], msk)
            nc.gpsimd.dma_start(ident_sb[:], ident)

            def rope(dst, raw_ps, pos_sl):
                """dst[128,512] <- RoPE(raw_ps[128,512] psum), via bf16 sbuf."""
                raw = psb.tile([128, 512], BF16, tag="rraw")
                nc.scalar.copy(raw[:], raw_ps[:])
                swp = psb.tile([128, 512], BF16, tag="rswp")
                # partition-half swap via SBUF->SBUF DMA; triggered on the
                # otherwise-idle Sync sequencer so it never queues behind
                # the 1MB xt loads on gpsimd.
                nc.sync.dma_start(swp[0:64, :], raw[64:128, :])
                nc.sync.dma_start(swp[64:128, :], raw[0:64, :])
                t1 = psb.tile([128, 512], BF16, tag="rt1")
                nc.vector.tensor_mul(t1[:], raw[:], cos_sb[:, pos_sl])
                t2 = psb.tile([128, 512], BF16, tag="rt2")
                nc.vector.tensor_mul(t2[:], swp[:], sin_sb[:, pos_sl])
                nc.vector.tensor_add(dst, t1[:], t2[:])

            for si in range(T // 512):
                b, sl = divmod(si, NSL)
                pos_sl = bass.ts(sl, 512)
                xt_sl = xt_tiles[si]
                if si >= 2:
                    # two 1MB chunks per slice (each splits across 16 SDMA
                    # engines internally)
                    for oc in range(2):
                        osl = slice(8 * oc, 8 * oc + 8)
                        (nc.gpsimd if oc == 0 else nc.scalar).dma_start(
                            xt_sl[:, osl, :], xt[:, si, osl, :]
                        )
                def do_q():
                    for h in range(HL):
                        q_ps = pps.tile([128, 512], F32, tag="q", name="q_ps",
                                        bufs=q_bufs)
                        for o in range(KD):
                            nc.tensor.matmul(
                                q_ps[:], wq_sb[:, o, bass.ts(h, DH)],
                                xt_sl[:, o, :],
                                start=(o == 0), stop=(o == KD - 1),
                            )
                        rope(qT_sb[:, h, b, pos_sl], q_ps, pos_sl)

                def do_kv():
                    k_ps = pps.tile([128, 512], F32, tag="k", name="k_ps",
                                    bufs=k_bufs)
                    for o in range(KD):
                        nc.tensor.matmul(
                            k_ps[:], wk_sb[:, o, :], xt_sl[:, o, :],
                            start=(o == 0), stop=(o == KD - 1),
                        )
                    rope(kT_sb[:, b, pos_sl], k_ps, pos_sl)
                    for jt in range(4):
                        v_ps = pps.tile([128, DH], F32, tag="v", name="v_ps",
                                        bufs=v_bufs)
                        for o in range(KD):
                            nc.tensor.matmul(
                                v_ps[:], xt_sl[:, o, bass.ts(jt, 128)],
                                wv_sb[:, o, :],
                                start=(o == 0), stop=(o == KD - 1),
                            )
                        nc.scalar.copy(v_sb[:, b, 4 * sl + jt, :], v_ps[:])

                if si == T // 512 - 1:
                    # last slice: q first so the phase tail is the short
                    # v-copy chain (ACT only) instead of the q-RoPE chain
                    # (ACT copy -> swap DMA -> DVE muls, ~4us), which
                    # otherwise stalls the first attention matmuls on the
                    # PSUM-bank handoff.
                    do_q()
                    do_kv()
                else:
                    do_kv()
                    do_q()

        # ---------------- phase C: attention + wo ----------------
        with tc.tile_pool(name="att_sb", bufs=asb_bufs) as asb, \
             tc.tile_pool(name="att_sb2", bufs=2) as asb2, \
             tc.tile_pool(name="att_ps", bufs=2, space="PSUM") as aps:

            def emit_wo_block(b, qs, nt, aoT, final):
                # wo + store for one 128-token row block; one [128, 4, 512]
                # f32 SBUF tile written as a single 1MB DMA (final blocks:
                # per-od 256KB DMAs so the drain starts per-copy).
                o_blk = asb.tile([128, 4, 512], F32, tag="out",
                                 bufs=out_bufs)
                row0 = b * S + qs * 512 + nt * 128
                for od in range(4):
                    w_ps = aps.tile([128, 512], F32, tag="wo", bufs=wo_bufs)
                    for c in range(HL):
                        nc.tensor.matmul(
                            w_ps[:], aoT[:, c, bass.ts(nt, 128)],
                            wo_sb[:, c, bass.ts(od, 512)],
                            start=(c == 0), stop=(c == HL - 1),
                        )
                    use_act = (
                        wo_copy == "act"
                        or (wo_copy == "mix" and (nt + od) % 2 == 0)
                    )
                    if use_act:
                        nc.scalar.copy(o_blk[:, od, :], w_ps[:])
                    else:
                        nc.vector.tensor_copy(o_blk[:, od, :], w_ps[:])
                    if final:
                        nc.sync.dma_start(
                            out[bass.ds(row0, 128), od, :], o_blk[:, od, :]
                        )
                if not final:
                    nc.sync.dma_start(out[bass.ds(row0, 128), :, :], o_blk[:])

            # wo blocks for iteration i are emitted one-per-head during
            # iteration i+1's attention loop, so the PE FIFO interleaves wo
            # matmuls with attention matmuls and each fills the other's
            # stall windows (exp latency vs psum-copy latency).
            pending_wo = []
            for b in range(BL):
                for qs in range(NSL):
                    q_sl = bass.ts(qs, 512)
                    nk = 4 * qs + 4
                    aoT = asb2.tile([128, HL, 512], BF16, tag="aoT")
                    for h in range(HL):
                        oT_ps = aps.tile([128, 512], F32, tag="oT", bufs=oT_bufs)
                        # bufs=1: keeps sc3+oT2+l1+wo2 = 8 PSUM banks
                        l_ps = aps.tile(
                            [128, 512], F32, tag="l", name="l_ps", bufs=1
                        )
                        l_started = False
                        e_quad = []
                        e_diag = []
                        for j in range(nk):
                            r = j - 4 * qs
                            masked = r >= 0
                            # columns q_local < 128 r are fully masked for
                            # the r-th diagonal tile: skip them everywhere
                            # (scores, mask, exp, PV, sums).
                            qlo = 128 * r if masked else 0
                            qg = bass.ds(qs * 512 + qlo, 512 - qlo)
                            s_ps = aps.tile([128, 512], F32, tag="sc", bufs=sc_bufs)
                            nc.tensor.matmul(
                                s_ps[:, qlo:], kT_sb[:, b, bass.ts(j, 128)],
                                qT_sb[:, h, b, qg], start=True,
                                stop=not masked,
                            )
                            if masked:
                                # scores += -30000 on causally-invalid
                                # entries; only the 128-col diagonal block
                                # can contain them, so narrow the matmul.
                                nc.tensor.matmul(
                                    s_ps[:, qlo:qlo + 128], ident_sb[:],
                                    msk_sb[:, r, qlo:qlo + 128],
                                    start=False, stop=True,
                                )
                            e_sb = asb.tile([128, 512], BF16, tag="exp", bufs=exp_bufs)
                            nc.scalar.activation(
                                e_sb[:, qlo:], s_ps[:, qlo:],
                                mybir.ActivationFunctionType.Exp, scale=scale,
                            )
                            nc.tensor.matmul(
                                oT_ps[:, qlo:], v_sb[:, b, j, :], e_sb[:, qlo:],
                                start=(j == 0), stop=(j == nk - 1),
                                skip_group_check=True,
                            )
                            if masked and l_quad:
                                # collect the 4 diagonal exp tiles; their l
                                # contribution is emitted range-batched
                                # after the j loop (see below).
                                e_diag.append(e_sb)
                            elif masked or not l_quad:
                                nc.tensor.matmul(
                                    l_ps[:, qlo:], ones_sb[:], e_sb[:, qlo:],
                                    start=not l_started, stop=(j == nk - 1),
                                    skip_group_check=True,
                                )
                                l_started = True
                            else:
                                # off-diagonal (full 512 cols): batch 4-8
                                # exp tiles into one ones-matmul via a DVE
                                # pairwise tree-sum (bf16, 2 elem/cyc/lane).
                                # group: 8 when 8+ off-diag tiles remain,
                                # else 4 (off-diag count is 4*qs).
                                e_quad.append(e_sb)
                                left = 4 * qs - (j + 1)
                                if (len(e_quad) == 8
                                        or (len(e_quad) == 4 and left < 4)):
                                    cur = e_quad
                                    while len(cur) > 1:
                                        nxt = []
                                        for a2, b2 in zip(cur[::2], cur[1::2]):
                                            t = asb.tile([128, 512], BF16,
                                                         tag="lt", bufs=8)
                                            nc.vector.tensor_add(
                                                t[:], a2[:], b2[:])
                                            nxt.append(t)
                                        cur = nxt
                                    nc.tensor.matmul(
                                        l_ps[:], ones_sb[:], cur[0][:],
                                        start=not l_started, stop=False,
                                        skip_group_check=True,
                                    )
                                    l_started = True
                                    e_quad = []
                        if e_diag:
                            # diagonal l, range-batched: the r-th diagonal
                            # tile only contributes for q >= 128r, so
                            #   l[0:128]   = e0            (overwrite range)
                            #   l[128:512] += e0+e1        (DVE pre-sum)
                            #   l[256:384] += e2
                            #   l[384:512] += e2+e3        (DVE pre-sum)
                            # 768 PE cycles instead of 1280; per-element
                            # has_written bits make the mixed overwrite/
                            # accumulate ranges compose.
                            e0, e1, e2, e3 = e_diag
                            da = asb.tile([128, 512], BF16, tag="lda", bufs=2)
                            nc.vector.tensor_add(
                                da[:, 128:], e0[:, 128:], e1[:, 128:])
                            db = asb.tile([128, 512], BF16, tag="ldb", bufs=2)
                            nc.vector.tensor_add(
                                db[:, 384:], e2[:, 384:], e3[:, 384:])
                            nc.tensor.matmul(
                                l_ps[:, 0:128], ones_sb[:], e0[:, 0:128],
                                start=not l_started, stop=False,
                                skip_group_check=True,
                            )
                            nc.tensor.matmul(
                                l_ps[:, 128:], ones_sb[:], da[:, 128:],
                                start=False, stop=False,
                                skip_group_check=True,
                            )
                            nc.tensor.matmul(
                                l_ps[:, 256:384], ones_sb[:],
                                e2[:, 256:384],
                                start=False, stop=False,
                                skip_group_check=True,
                            )
                            nc.tensor.matmul(
                                l_ps[:, 384:], ones_sb[:], db[:, 384:],
                                start=False, stop=True,
                                skip_group_check=True,
                            )
                        rc_sb = asb.tile([128, 512], F32, tag="rc")
                        nc.vector.reciprocal_approx_fast(rc_sb[:], l_ps[:])
                        nc.vector.tensor_mul(aoT[:, h, :], oT_ps[:], rc_sb[:])
                        if pending_wo:
                            pending_wo.pop(0)()
                    final = (b == BL - 1 and qs == NSL - 1)
                    for nt in range(4):
                        pending_wo.append(
                            (lambda b=b, qs=qs, nt=nt, aoT=aoT, f=final:
                             emit_wo_block(b, qs, nt, aoT, f))
                        )
            for go in pending_wo:
                go()
    nc.finalize()
    return nc


_NC_CACHE = {}


def _get_nc():
    if "nc" not in _NC_CACHE:
        _NC_CACHE["nc"] = build_nc()
    return _NC_CACHE["nc"]


def kernel(x, freqs_cos, freqs_sin, wq, wk, wv, wo):
    x = np.asarray(x)
    freqs_cos = np.asarray(freqs_cos)
    freqs_sin = np.asarray(freqs_sin)
    wq = np.asarray(wq)
    wk = np.asarray(wk)
    wv = np.asarray(wv)
    wo = np.asarray(wo)
    bf = ml_dtypes.bfloat16
    perm = np.concatenate([np.arange(0, DH, 2), np.arange(1, DH, 2)])

    wq_p = wq.reshape(D, NH, DH)[:, :, perm].reshape(D, NH * DH)
    wk_p = wk.reshape(D, NKV, DH)[:, :, perm].reshape(D, NKV * DH)

    cosT = freqs_cos.T  # [64, S]
    sinT = freqs_sin.T
    c2 = np.ascontiguousarray(np.concatenate([cosT, cosT], axis=0)).astype(bf)
    s2 = np.ascontiguousarray(np.concatenate([-sinT, sinT], axis=0)).astype(bf)

    # additive mask[k, r, q] = 0 if causally valid (q - k - 128 r >= 0),
    # else -30000 (exp underflows to exactly 0 after 1/sqrt(dh) scaling)
    kk = np.arange(128)[:, None, None]
    rr = np.arange(4)[None, :, None]
    qq = np.arange(512)[None, None, :]
    masks = np.ascontiguousarray(
        np.where(qq - kk - 128 * rr >= 0, 0.0, -30000.0)
    ).astype(bf)
    identity = np.eye(128, dtype=np.float32).astype(bf)

    def tile_weight(w, dt):
        # [D, C] -> [128, KD, C] with w[o*128+p, c] -> [p, o, c], contiguous
        c = w.shape[1]
        return np.ascontiguousarray(
            w.reshape(KD, 128, c).transpose(1, 0, 2)
        ).astype(dt)

    in_maps = []
    for core in range(8):
        dp, tp = divmod(core, TP)
        xs = x[dp * BL: (dp + 1) * BL].reshape(T, D)
        # [D, T] -> [128, T//512, KD, 512]: x.T[o*128+p, sl*512+t] ->
        # [p, sl, o, t], fully contiguous per partition line
        xt = np.ascontiguousarray(
            xs.T.reshape(KD, 128, T // 512, 512).transpose(1, 2, 0, 3)
        ).astype(bf)
        wo_sl = wo[tp * QC: (tp + 1) * QC, :]
        wo_t = np.ascontiguousarray(
            wo_sl.reshape(HL, 128, D).transpose(1, 0, 2)
        ).astype(bf)
        in_maps.append(
            {
                "xt": xt,
                "wq": tile_weight(wq_p[:, tp * QC: (tp + 1) * QC], bf),
                "wk": tile_weight(wk_p[:, tp * DH: (tp + 1) * DH], bf),
                "wv": tile_weight(wv[:, tp * DH: (tp + 1) * DH], bf),
                "wo": wo_t,
                "cos2": c2,
                "sin2": s2,
                "msk": masks,
                "ident": identity,
            }
        )

    nc = _get_nc()
    res = run_bass_kernel_spmd(nc, in_maps, core_ids=list(range(8)))
    _NC_CACHE["last_results"] = res

    full = np.zeros((B, S, D), dtype=np.float32)
    for core in range(8):
        dp = core // TP
        full[dp * BL: (dp + 1) * BL] += (
            res.results[core]["out"].astype(np.float32).reshape(BL, S, D)
        )
    return full


# revision 26
# speedup vs baseline: 1.0329x; 1.0329x over previous
"""GQA attention layer (B=4, S=2048, D=2048, 16 heads / 4 KV heads, RoPE,
causal) on 8 trn2 NeuronCores.

Sharding: TP=4 over KV-head groups x DP=2 over batch. Each core handles 2
batches and one KV group (4 q heads + 1 kv head), computes a partial
(head-group) contribution to out = attn @ wo; host sums the 4 partials per
batch group.

Device layout choices:
  - host pre-casts to bf16 and pre-tiles everything into [128, ...]
    partition-contiguous layouts (pure per-partition memcpy DMAs), and
    permutes wq/wk columns per head to "evens then odds" so RoPE becomes
    rotate-half.
  - DMA triggers are spread across the idle Tensor/Vector/GpSimd sequencer
    queues (each DIRECT2D trigger costs ~0.6us of sequencer time; the Sync
    queue alone serializes them). wk/wv are loaded first and slice 0 runs
    k/v projections before q so the PE can start ~2us in on the small
    weights while wq streams.
  - q,k are produced transposed ([dh, tok]) straight from the projection
    matmuls; v is produced natural ([tok, dh]).
  - RoPE in transposed layout: rot = q * C2 + swap_halves(q) * S2 with
    C2 = [cos;cos], S2 = [-sin;+sin]; the half swap is 2 SBUF->SBUF DMAs.
  - attention: scoresT = kT_tile.T @ qT (k on partitions), exp on ACT (no
    max subtraction -- scores are O(5) here). Causal masking stays on the
    Tensor engine: diagonal tiles get -30000 added via an extra
    identity @ mask matmul (narrowed to the 128-col diagonal block)
    accumulated into the scores PSUM group, and all ops on a diagonal tile
    skip its fully-masked first 128*r columns.
    PV matmuls accumulate in PSUM over k-tiles. The softmax denominator
    ones-matmul is quad-batched for off-diagonal tiles: 4 exp tiles are
    tree-summed on DVE (3 bf16 adds) and a single ones @ esum matmul
    accumulates into l (4x fewer PE cycles); diagonal tiles keep per-tile
    ones-matmuls narrowed by qlo. 1/l via DVE reciprocal_approx_fast.
  - wo outputs for one 128-token row block are collected into a single
    [128, 4, 512] f32 SBUF tile and written with ONE 1MB DMA (contiguous
    8KB per partition line, split across all 16 SDMA engines) triggered on
    the idle Sync sequencer: 32 triggers total instead of 256 on GpSimd,
    which removes the out-DMA completion stalls and the serialized
    trigger tail after the last matmul.
  - wo blocks for iteration (b, qs) are emitted one-per-head during
    iteration (b, qs)+1's attention loop, so the PE FIFO interleaves wo
    matmuls with attention matmuls: exp-latency stalls are filled by wo
    work and psum-copy stalls by attention work. This also keeps the PE
    HAM clock-gate warm (K=8/8) through all of phase C.
"""

import itertools
import math
from contextlib import ExitStack

import ml_dtypes
import numpy as np

import concourse.bass as bass
import concourse.mybir as mybir
import concourse.tile as tile
from concourse import bacc
from concourse.bass_utils import run_bass_kernel_spmd

BF16 = mybir.dt.bfloat16
F32 = mybir.dt.float32

# Full-problem constants (hardcoded per harness contract)
B, S, D = 4, 2048, 2048
NH, NKV, DH = 16, 4, 128
TP, DP = 4, 2
BL = B // DP          # batches per core
T = BL * S            # tokens per core
HL = NH // TP         # q heads per core
QC = HL * DH          # q cols per core
NT128 = S // 128      # 128-token tiles per batch (16)
NSL = S // 512        # 512-token slices per batch (4)
KD = D // 128         # contraction tiles for the projections (16)


def build_nc(sc_bufs=3, oT_bufs=2, wo_bufs=2, exp_bufs=10, xt_bufs=2,
             q_bufs=2, k_bufs=2, v_bufs=2, wo_copy="mix",
             psb_bufs=3, asb_bufs=4, out_bufs=3, l_quad=True):
    nc = bacc.Bacc("TRN2", target_bir_lowering=False, debug=False)

    xt = nc.dram_tensor("xt", [128, T // 512, KD, 512], BF16,
                        kind="ExternalInput").ap()
    wq = nc.dram_tensor("wq", [128, KD, QC], BF16, kind="ExternalInput").ap()
    wk = nc.dram_tensor("wk", [128, KD, DH], BF16, kind="ExternalInput").ap()
    wv = nc.dram_tensor("wv", [128, KD, DH], BF16, kind="ExternalInput").ap()
    wo = nc.dram_tensor("wo", [128, HL, D], BF16, kind="ExternalInput").ap()
    cos2 = nc.dram_tensor("cos2", [128, S], BF16, kind="ExternalInput").ap()
    sin2 = nc.dram_tensor("sin2", [128, S], BF16, kind="ExternalInput").ap()
    msk = nc.dram_tensor("msk", [128, 4, 512], BF16, kind="ExternalInput").ap()
    ident = nc.dram_tensor("ident", [128, 128], BF16, kind="ExternalInput").ap()
    # [T, D] viewed as [T, 4, 512] so a [128, 4, 512] SBUF tile DMAs as one
    # fully-contiguous 1MB transfer per 128-token row block.
    out = nc.dram_tensor("out", [T, 4, 512], F32, kind="ExternalOutput").ap()

    scale = 1.0 / math.sqrt(DH)

    with tile.TileContext(nc) as tc, ExitStack() as ctx:
        persist = ctx.enter_context(tc.tile_pool(name="persist", bufs=1))

        # --- resident weights / tables ---
        wq_sb = persist.tile([128, KD, QC], BF16, tag="wq")
        wk_sb = persist.tile([128, KD, DH], BF16, tag="wk")
        wv_sb = persist.tile([128, KD, DH], BF16, tag="wv")
        wo_sb = persist.tile([128, HL, D], BF16, tag="wo")
        cos_sb = persist.tile([128, S], BF16, tag="cos")
        sin_sb = persist.tile([128, S], BF16, tag="sin")
        msk_sb = persist.tile([128, 4, 512], BF16, tag="msk")
        ident_sb = persist.tile([128, 128], BF16, tag="ident")
        ones_sb = persist.tile([128, 128], BF16, tag="ones")
        nc.vector.memset(ones_sb[:], 1.0)

        # --- resident activations ---
        qT_sb = persist.tile([128, HL, BL, S], BF16, tag="qT")
        kT_sb = persist.tile([128, BL, S], BF16, tag="kT")
        v_sb = persist.tile([128, BL, NT128, DH], BF16, tag="v")

        # ---------------- phase B: projections + RoPE ----------------
        with tc.tile_pool(name="proj_sb", bufs=psb_bufs) as psb, \
             tc.tile_pool(name="proj_ps", bufs=2, space="PSUM") as pps:

            # Pre-allocate the xt tiles for all slices up-front so slice 0's
            # load can be emitted first with fine-grained chunking.
            xt_tiles = [
                psb.tile([128, KD, 512], BF16, tag="xt", name="xt_sl",
                         bufs=xt_bufs)
                for _ in range(T // 512)
            ]

            # Highest priority: wk/wv (small, unblock k/v projections fast),
            # then xt slice 0 and wq in 4-o chunks (512KB: each DMA already
            # splits across all 16 SDMA engines, so fewer/bigger triggers
            # beat fine chunking -- each DIRECT2D trigger costs ~0.6us of
            # sequencer time), then slice 1. RoPE tables go early on sync
            # (cos/sin are needed by slice 0's rope); the phase-C tables
            # ride on gpsimd behind slice 1.
            # Small chunks are slow per-transfer (descriptor-dominated once
            # split 16 ways), so what matters is spreading them round-robin
            # across the three trigger queues so several are in flight at
            # once -- NOT dependency-ordering them on one queue.
            nc.scalar.dma_start(wk_sb[:], wk)
            nc.gpsimd.dma_start(wv_sb[:], wv)
            trig = itertools.cycle([nc.sync, nc.scalar, nc.gpsimd])
            for osl in (slice(0, 2), slice(2, 4), slice(4, 6), slice(6, 8),
                        slice(8, 12), slice(12, 16)):
                next(trig).dma_start(xt_tiles[0][:, osl, :], xt[:, 0, osl, :])
            for oc in range(4):
                osl = slice(4 * oc, 4 * oc + 4)
                next(trig).dma_start(wq_sb[:, osl, :], wq[:, osl, :])
            nc.sync.dma_start(cos_sb[:], cos2)
            nc.sync.dma_start(sin_sb[:], sin2)
            for oc in range(2):
                osl = slice(8 * oc, 8 * oc + 8)
                next(trig).dma_start(xt_tiles[1][:, osl, :], xt[:, 1, osl, :])
            for c in range(HL):
                nc.gpsimd.dma_start(wo_sb[:, c, :], wo[:, c, :])
            nc.gpsimd.dma_start(msk_sb[:], msk)
            nc.gpsimd.dma_start(ident_sb[:], ident)

            def rope(dst, raw_ps, pos_sl):
                """dst[128,512] <- RoPE(raw_ps[128,512] psum), via bf16 sbuf."""
                raw = psb.tile([128, 512], BF16, tag="rraw")
                nc.scalar.copy(raw[:], raw_ps[:])
                swp = psb.tile([128, 512], BF16, tag="rswp")
                # partition-half swap via SBUF->SBUF DMA; triggered on the
                # otherwise-idle Sync sequencer so it never queues behind
                # the 1MB xt loads on gpsimd.
                nc.sync.dma_start(swp[0:64, :], raw[64:128, :])
                nc.sync.dma_start(swp[64:128, :], raw[0:64, :])
                t1 = psb.tile([128, 512], BF16, tag="rt1")
                nc.vector.tensor_mul(t1[:], raw[:], cos_sb[:, pos_sl])
                t2 = psb.tile([128, 512], BF16, tag="rt2")
                nc.vector.tensor_mul(t2[:], swp[:], sin_sb[:, pos_sl])
                nc.vector.tensor_add(dst, t1[:], t2[:])

            for si in range(T // 512):
                b, sl = divmod(si, NSL)
                pos_sl = bass.ts(sl, 512)
                xt_sl = xt_tiles[si]
                if si >= 2:
                    # two 1MB chunks per slice (each splits across 16 SDMA
                    # engines internally)
                    for oc in range(2):
                        osl = slice(8 * oc, 8 * oc + 8)
                        (nc.gpsimd if oc == 0 else nc.scalar).dma_start(
                            xt_sl[:, osl, :], xt[:, si, osl, :]
                        )
                def do_q():
                    for h in range(HL):
                        q_ps = pps.tile([128, 512], F32, tag="q", name="q_ps",
                                        bufs=q_bufs)
                        for o in range(KD):
                            nc.tensor.matmul(
                                q_ps[:], wq_sb[:, o, bass.ts(h, DH)],
                                xt_sl[:, o, :],
                                start=(o == 0), stop=(o == KD - 1),
                            )
                        rope(qT_sb[:, h, b, pos_sl], q_ps, pos_sl)

                def do_kv():
                    k_ps = pps.tile([128, 512], F32, tag="k", name="k_ps",
                                    bufs=k_bufs)
                    for o in range(KD):
                        nc.tensor.matmul(
                            k_ps[:], wk_sb[:, o, :], xt_sl[:, o, :],
                            start=(o == 0), stop=(o == KD - 1),
                        )
                    rope(kT_sb[:, b, pos_sl], k_ps, pos_sl)
                    for jt in range(4):
                        v_ps = pps.tile([128, DH], F32, tag="v", name="v_ps",
                                        bufs=v_bufs)
                        for o in range(KD):
                            nc.tensor.matmul(
                                v_ps[:], xt_sl[:, o, bass.ts(jt, 128)],
                                wv_sb[:, o, :],
                                start=(o == 0), stop=(o == KD - 1),
                            )
                        nc.scalar.copy(v_sb[:, b, 4 * sl + jt, :], v_ps[:])

                if si == T // 512 - 1:
                    # last slice: q first so the phase tail is the short
                    # v-copy chain (ACT only) instead of the q-RoPE chain
                    # (ACT copy -> swap DMA -> DVE muls, ~4us), which
                    # otherwise stalls the first attention matmuls on the
                    # PSUM-bank handoff.
                    do_q()
                    do_kv()
                else:
                    do_kv()
                    do_q()

        # ---------------- phase C: attention + wo ----------------
        with tc.tile_pool(name="att_sb", bufs=asb_bufs) as asb, \
             tc.tile_pool(name="att_sb2", bufs=2) as asb2, \
             tc.tile_pool(name="att_ps", bufs=2, space="PSUM") as aps:

            def emit_wo_block(b, qs, nt, aoT, final):
                # wo + store for one 128-token row block; one [128, 4, 512]
                # f32 SBUF tile written as a single 1MB DMA (final blocks:
                # per-od 256KB DMAs so the drain starts per-copy).
                o_blk = asb.tile([128, 4, 512], F32, tag="out",
                                 bufs=out_bufs)
                row0 = b * S + qs * 512 + nt * 128
                for od in range(4):
                    w_ps = aps.tile([128, 512], F32, tag="wo", bufs=wo_bufs)
                    for c in range(HL):
                        nc.tensor.matmul(
                            w_ps[:], aoT[:, c, bass.ts(nt, 128)],
                            wo_sb[:, c, bass.ts(od, 512)],
                            start=(c == 0), stop=(c == HL - 1),
                        )
                    use_act = (
                        wo_copy == "act"
                        or (wo_copy == "mix" and (nt + od) % 2 == 0)
                    )
                    if use_act:
                        nc.scalar.copy(o_blk[:, od, :], w_ps[:])
                    else:
                        nc.vector.tensor_copy(o_blk[:, od, :], w_ps[:])
                    if final:
                        nc.sync.dma_start(
                            out[bass.ds(row0, 128), od, :], o_blk[:, od, :]
                        )
                if not final:
                    nc.sync.dma_start(out[bass.ds(row0, 128), :, :], o_blk[:])

            # wo blocks for iteration i are emitted one-per-head during
            # iteration i+1's attention loop, so the PE FIFO interleaves wo
            # matmuls with attention matmuls and each fills the other's
            # stall windows (exp latency vs psum-copy latency).
            pending_wo = []
            for b in range(BL):
                for qs in range(NSL):
                    q_sl = bass.ts(qs, 512)
                    nk = 4 * qs + 4
                    aoT = asb2.tile([128, HL, 512], BF16, tag="aoT")
                    for h in range(HL):
                        oT_ps = aps.tile([128, 512], F32, tag="oT", bufs=oT_bufs)
                        # bufs=1: keeps sc3+oT2+l1+wo2 = 8 PSUM banks
                        l_ps = aps.tile(
                            [128, 512], F32, tag="l", name="l_ps", bufs=1
                        )
                        l_started = False
                        e_quad = []
                        e_diag = []
                        for j in range(nk):
                            r = j - 4 * qs
                            masked = r >= 0
                            # columns q_local < 128 r are fully masked for
                            # the r-th diagonal tile: skip them everywhere
                            # (scores, mask, exp, PV, sums).
                            qlo = 128 * r if masked else 0
                            qg = bass.ds(qs * 512 + qlo, 512 - qlo)
                            s_ps = aps.tile([128, 512], F32, tag="sc", bufs=sc_bufs)
                            nc.tensor.matmul(
                                s_ps[:, qlo:], kT_sb[:, b, bass.ts(j, 128)],
                                qT_sb[:, h, b, qg], start=True,
                                stop=not masked,
                            )
                            if masked:
                                # scores += -30000 on causally-invalid
                                # entries; only the 128-col diagonal block
                                # can contain them, so narrow the matmul.
                                nc.tensor.matmul(
                                    s_ps[:, qlo:qlo + 128], ident_sb[:],
                                    msk_sb[:, r, qlo:qlo + 128],
                                    start=False, stop=True,
                                )
                            e_sb = asb.tile([128, 512], BF16, tag="exp", bufs=exp_bufs)
                            nc.scalar.activation(
                                e_sb[:, qlo:], s_ps[:, qlo:],
                                mybir.ActivationFunctionType.Exp, scale=scale,
                            )
                            nc.tensor.matmul(
                                oT_ps[:, qlo:], v_sb[:, b, j, :], e_sb[:, qlo:],
                                start=(j == 0), stop=(j == nk - 1),
                                skip_group_check=True,
                            )
                            if masked and l_quad:
                                # collect the 4 diagonal exp tiles; their l
                                # contribution is emitted range-batched
                                # after the j loop (see below).
                                e_diag.append(e_sb)
                            elif masked or not l_quad:
                                nc.tensor.matmul(
                                    l_ps[:, qlo:], ones_sb[:], e_sb[:, qlo:],
                                    start=not l_started, stop=(j == nk - 1),
                                    skip_group_check=True,
                                )
                                l_started = True
                            else:
                                # off-diagonal (full 512 cols): batch 4-8
                                # exp tiles into one ones-matmul via a DVE
                                # pairwise tree-sum (bf16, 2 elem/cyc/lane).
                                # group: 8 when 8+ off-diag tiles remain,
                                # else 4 (off-diag count is 4*qs).
                                e_quad.append(e_sb)
                                left = 4 * qs - (j + 1)
                                if (len(e_quad) == 8
                                        or (len(e_quad) == 4 and left < 4)):
                                    cur = e_quad
                                    while len(cur) > 1:
                                        nxt = []
                                        for a2, b2 in zip(cur[::2], cur[1::2]):
                                            t = asb.tile([128, 512], BF16,
                                                         tag="lt", bufs=8)
                                            nc.vector.tensor_add(
                                                t[:], a2[:], b2[:])
                                            nxt.append(t)
                                        cur = nxt
                                    nc.tensor.matmul(
                                        l_ps[:], ones_sb[:], cur[0][:],
                                        start=not l_started, stop=False,
                                        skip_group_check=True,
                                    )
                                    l_started = True
                                    e_quad = []
                        if e_diag:
                            # diagonal l, range-batched: the r-th diagonal
                            # tile only contributes for q >= 128r, so
                            #   l[0:128]   = e0            (overwrite range)
                            #   l[128:512] += e0+e1        (DVE pre-sum)
                            #   l[256:384] += e2
                            #   l[384:512] += e2+e3        (DVE pre-sum)
                            # 768 PE cycles instead of 1280; per-element
                            # has_written bits make the mixed overwrite/
                            # accumulate ranges compose.
                            e0, e1, e2, e3 = e_diag
                            da = asb.tile([128, 512], BF16, tag="lda", bufs=2)
                            nc.vector.tensor_add(
                                da[:, 128:], e0[:, 128:], e1[:, 128:])
                            db = asb.tile([128, 512], BF16, tag="ldb", bufs=2)
                            nc.vector.tensor_add(
                                db[:, 384:], e2[:, 384:], e3[:, 384:])
                            nc.tensor.matmul(
                                l_ps[:, 0:128], ones_sb[:], e0[:, 0:128],
                                start=not l_started, stop=False,
                                skip_group_check=True,
                            )
                            nc.tensor.matmul(
                                l_ps[:, 128:], ones_sb[:], da[:, 128:],
                                start=False, stop=False,
                                skip_group_check=True,
                            )
                            nc.tensor.matmul(
                                l_ps[:, 256:384], ones_sb[:],
                                e2[:, 256:384],
                                start=False, stop=False,
                                skip_group_check=True,
                            )
                            nc.tensor.matmul(
                                l_ps[:, 384:], ones_sb[:], db[:, 384:],
                                start=False, stop=True,
                                skip_group_check=True,
                            )
                        rc_sb = asb.tile([128, 512], F32, tag="rc")
                        nc.vector.reciprocal_approx_fast(rc_sb[:], l_ps[:])
                        nc.vector.tensor_mul(aoT[:, h, :], oT_ps[:], rc_sb[:])
                        if pending_wo:
                            pending_wo.pop(0)()
                    final = (b == BL - 1 and qs == NSL - 1)
                    for nt in range(4):
                        pending_wo.append(
                            (lambda b=b, qs=qs, nt=nt, aoT=aoT, f=final:
                             emit_wo_block(b, qs, nt, aoT, f))
                        )
            for go in pending_wo:
                go()
    nc.finalize()
    return nc


_NC_CACHE = {}


def _get_nc():
    if "nc" not in _NC_CACHE:
        _NC_CACHE["nc"] = build_nc()
    return _NC_CACHE["nc"]


def kernel(x, freqs_cos, freqs_sin, wq, wk, wv, wo):
    x = np.asarray(x)
    freqs_cos = np.asarray(freqs_cos)
    freqs_sin = np.asarray(freqs_sin)
    wq = np.asarray(wq)
    wk = np.asarray(wk)
    wv = np.asarray(wv)
    wo = np.asarray(wo)
    bf = ml_dtypes.bfloat16
    perm = np.concatenate([np.arange(0, DH, 2), np.arange(1, DH, 2)])

    wq_p = wq.reshape(D, NH, DH)[:, :, perm].reshape(D, NH * DH)
    wk_p = wk.reshape(D, NKV, DH)[:, :, perm].reshape(D, NKV * DH)

    cosT = freqs_cos.T  # [64, S]
    sinT = freqs_sin.T
    c2 = np.ascontiguousarray(np.concatenate([cosT, cosT], axis=0)).astype(bf)
    s2 = np.ascontiguousarray(np.concatenate([-sinT, sinT], axis=0)).astype(bf)

    # additive mask[k, r, q] = 0 if causally valid (q - k - 128 r >= 0),
    # else -30000 (exp underflows to exactly 0 after 1/sqrt(dh) scaling)
    kk = np.arange(128)[:, None, None]
    rr = np.arange(4)[None, :, None]
    qq = np.arange(512)[None, None, :]
    masks = np.ascontiguousarray(
        np.where(qq - kk - 128 * rr >= 0, 0.0, -30000.0)
    ).astype(bf)
    identity = np.eye(128, dtype=np.float32).astype(bf)

    def tile_weight(w, dt):
        # [D, C] -> [128, KD, C] with w[o*128+p, c] -> [p, o, c], contiguous
        c = w.shape[1]
        return np.ascontiguousarray(
            w.reshape(KD, 128, c).transpose(1, 0, 2)
        ).astype(dt)

    in_maps = []
    for core in range(8):
        dp, tp = divmod(core, TP)
        xs = x[dp * BL: (dp + 1) * BL].reshape(T, D)
        # [D, T] -> [128, T//512, KD, 512]: x.T[o*128+p, sl*512+t] ->
        # [p, sl, o, t], fully contiguous per partition line
        xt = np.ascontiguousarray(
            xs.T.reshape(KD, 128, T // 512, 512).transpose(1, 2, 0, 3)
        ).astype(bf)
        wo_sl = wo[tp * QC: (tp + 1) * QC, :]
        wo_t = np.ascontiguousarray(
            wo_sl.reshape(HL, 128, D).transpose(1, 0, 2)
        ).astype(bf)
        in_maps.append(
            {
                "xt": xt,
                "wq": tile_weight(wq_p[:, tp * QC: (tp + 1) * QC], bf),
                "wk": tile_weight(wk_p[:, tp * DH: (tp + 1) * DH], bf),
                "wv": tile_weight(wv[:, tp * DH: (tp + 1) * DH], bf),
                "wo": wo_t,
                "cos2": c2,
                "sin2": s2,
                "msk": masks,
                "ident": identity,
            }
        )

    nc = _get_nc()
    res = run_bass_kernel_spmd(nc, in_maps, core_ids=list(range(8)))
    _NC_CACHE["last_results"] = res

    full = np.zeros((B, S, D), dtype=np.float32)
    for core in range(8):
        dp = core // TP
        full[dp * BL: (dp + 1) * BL] += (
            res.results[core]["out"].astype(np.float32).reshape(BL, S, D)
        )
    return full


# revision 28
# speedup vs baseline: 1.0349x; 1.0019x over previous
"""GQA attention layer (B=4, S=2048, D=2048, 16 heads / 4 KV heads, RoPE,
causal) on 8 trn2 NeuronCores.

Sharding: TP=4 over KV-head groups x DP=2 over batch. Each core handles 2
batches and one KV group (4 q heads + 1 kv head), computes a partial
(head-group) contribution to out = attn @ wo; host sums the 4 partials per
batch group.

Device layout choices:
  - host pre-casts to bf16 and pre-tiles everything into [128, ...]
    partition-contiguous layouts (pure per-partition memcpy DMAs), and
    permutes wq/wk columns per head to "evens then odds" so RoPE becomes
    rotate-half.
  - DMA triggers are spread across the idle Tensor/Vector/GpSimd sequencer
    queues (each DIRECT2D trigger costs ~0.6us of sequencer time; the Sync
    queue alone serializes them). wk/wv are loaded first and slice 0 runs
    k/v projections before q so the PE can start ~2us in on the small
    weights while wq streams.
  - q,k are produced transposed ([dh, tok]) straight from the projection
    matmuls; v is produced natural ([tok, dh]).
  - RoPE in transposed layout: rot = q * C2 + swap_halves(q) * S2 with
    C2 = [cos;cos], S2 = [-sin;+sin]; the half swap is 2 SBUF->SBUF DMAs.
  - attention: scoresT = kT_tile.T @ qT (k on partitions), exp on ACT (no
    max subtraction -- scores are O(5) here). Causal masking stays on the
    Tensor engine: diagonal tiles get -30000 added via an extra
    identity @ mask matmul (narrowed to the 128-col diagonal block)
    accumulated into the scores PSUM group, and all ops on a diagonal tile
    skip its fully-masked first 128*r columns.
    PV matmuls accumulate in PSUM over k-tiles. The softmax denominator
    ones-matmul is quad-batched for off-diagonal tiles: 4 exp tiles are
    tree-summed on DVE (3 bf16 adds) and a single ones @ esum matmul
    accumulates into l (4x fewer PE cycles); diagonal tiles keep per-tile
    ones-matmuls narrowed by qlo. 1/l via DVE reciprocal_approx_fast.
  - wo outputs for one 128-token row block are collected into a single
    [128, 4, 512] f32 SBUF tile and written with ONE 1MB DMA (contiguous
    8KB per partition line, split across all 16 SDMA engines) triggered on
    the idle Sync sequencer: 32 triggers total instead of 256 on GpSimd,
    which removes the out-DMA completion stalls and the serialized
    trigger tail after the last matmul.
  - wo blocks for iteration (b, qs) are emitted one-per-head during
    iteration (b, qs)+1's attention loop, so the PE FIFO interleaves wo
    matmuls with attention matmuls: exp-latency stalls are filled by wo
    work and psum-copy stalls by attention work. This also keeps the PE
    HAM clock-gate warm (K=8/8) through all of phase C.
"""

import itertools
import math
from contextlib import ExitStack

import ml_dtypes
import numpy as np

import concourse.bass as bass
import concourse.mybir as mybir
import concourse.tile as tile
from concourse import bacc
from concourse.bass_utils import run_bass_kernel_spmd

BF16 = mybir.dt.bfloat16
F32 = mybir.dt.float32

# Full-problem constants (hardcoded per harness contract)
B, S, D = 4, 2048, 2048
NH, NKV, DH = 16, 4, 128
TP, DP = 4, 2
BL = B // DP          # batches per core
T = BL * S            # tokens per core
HL = NH // TP         # q heads per core
QC = HL * DH          # q cols per core
NT128 = S // 128      # 128-token tiles per batch (16)
NSL = S // 512        # 512-token slices per batch (4)
KD = D // 128         # contraction tiles for the projections (16)


def build_nc(sc_bufs=3, oT_bufs=2, wo_bufs=2, exp_bufs=10, xt_bufs=2,
             q_bufs=2, k_bufs=2, v_bufs=2, wo_copy="mix",
             psb_bufs=3, asb_bufs=4, out_bufs=3, l_quad=True):
    nc = bacc.Bacc("TRN2", target_bir_lowering=False, debug=False)

    xt = nc.dram_tensor("xt", [128, T // 512, KD, 512], BF16,
                        kind="ExternalInput").ap()
    wq = nc.dram_tensor("wq", [128, KD, QC], BF16, kind="ExternalInput").ap()
    wk = nc.dram_tensor("wk", [128, KD, DH], BF16, kind="ExternalInput").ap()
    wv = nc.dram_tensor("wv", [128, KD, DH], BF16, kind="ExternalInput").ap()
    wo = nc.dram_tensor("wo", [128, HL, D], BF16, kind="ExternalInput").ap()
    cos2 = nc.dram_tensor("cos2", [128, S], BF16, kind="ExternalInput").ap()
    sin2 = nc.dram_tensor("sin2", [128, S], BF16, kind="ExternalInput").ap()
    msk = nc.dram_tensor("msk", [128, 4, 512], BF16, kind="ExternalInput").ap()
    ident = nc.dram_tensor("ident", [128, 128], BF16, kind="ExternalInput").ap()
    # [T, D] viewed as [T, 4, 512] so a [128, 4, 512] SBUF tile DMAs as one
    # fully-contiguous 1MB transfer per 128-token row block.
    out = nc.dram_tensor("out", [T, 4, 512], F32, kind="ExternalOutput").ap()

    scale = 1.0 / math.sqrt(DH)

    with tile.TileContext(nc) as tc, ExitStack() as ctx:
        persist = ctx.enter_context(tc.tile_pool(name="persist", bufs=1))

        # --- resident weights / tables ---
        wq_sb = persist.tile([128, KD, QC], BF16, tag="wq")
        wk_sb = persist.tile([128, KD, DH], BF16, tag="wk")
        wv_sb = persist.tile([128, KD, DH], BF16, tag="wv")
        wo_sb = persist.tile([128, HL, D], BF16, tag="wo")
        cos_sb = persist.tile([128, S], BF16, tag="cos")
        sin_sb = persist.tile([128, S], BF16, tag="sin")
        msk_sb = persist.tile([128, 4, 512], BF16, tag="msk")
        ident_sb = persist.tile([128, 128], BF16, tag="ident")
        ones_sb = persist.tile([128, 128], BF16, tag="ones")
        nc.vector.memset(ones_sb[:], 1.0)

        # --- resident activations ---
        qT_sb = persist.tile([128, HL, BL, S], BF16, tag="qT")
        kT_sb = persist.tile([128, BL, S], BF16, tag="kT")
        v_sb = persist.tile([128, BL, NT128, DH], BF16, tag="v")

        # ---------------- phase B: projections + RoPE ----------------
        with tc.tile_pool(name="proj_sb", bufs=psb_bufs) as psb, \
             tc.tile_pool(name="proj_ps", bufs=2, space="PSUM") as pps:

            # Pre-allocate the xt tiles for all slices up-front so slice 0's
            # load can be emitted first with fine-grained chunking.
            xt_tiles = [
                psb.tile([128, KD, 512], BF16, tag="xt", name="xt_sl",
                         bufs=xt_bufs)
                for _ in range(T // 512)
            ]

            # Highest priority: wk/wv (small, unblock k/v projections fast),
            # then xt slice 0 and wq in 4-o chunks (512KB: each DMA already
            # splits across all 16 SDMA engines, so fewer/bigger triggers
            # beat fine chunking -- each DIRECT2D trigger costs ~0.6us of
            # sequencer time), then slice 1. RoPE tables go early on sync
            # (cos/sin are needed by slice 0's rope); the phase-C tables
            # ride on gpsimd behind slice 1.
            # Small chunks are slow per-transfer (descriptor-dominated once
            # split 16 ways), so what matters is spreading them round-robin
            # across the three trigger queues so several are in flight at
            # once -- NOT dependency-ordering them on one queue.
            nc.scalar.dma_start(wk_sb[:], wk)
            nc.gpsimd.dma_start(wv_sb[:], wv)
            trig = itertools.cycle([nc.sync, nc.scalar, nc.gpsimd])
            for osl in (slice(0, 2), slice(2, 4), slice(4, 6), slice(6, 8),
                        slice(8, 12), slice(12, 16)):
                next(trig).dma_start(xt_tiles[0][:, osl, :], xt[:, 0, osl, :])
            for oc in range(4):
                osl = slice(4 * oc, 4 * oc + 4)
                next(trig).dma_start(wq_sb[:, osl, :], wq[:, osl, :])
            nc.sync.dma_start(cos_sb[:], cos2)
            nc.sync.dma_start(sin_sb[:], sin2)
            # xt slice 1 AFTER wq: the startup window is HBM-bw-bound, and
            # wq is needed ~15us in while slice 1 isn't touched until ~35us
            for oc in range(2):
                osl = slice(8 * oc, 8 * oc + 8)
                next(trig).dma_start(xt_tiles[1][:, osl, :], xt[:, 1, osl, :])

            def rope(dst, raw_ps, pos_sl):
                """dst[128,512] <- RoPE(raw_ps[128,512] psum), via bf16 sbuf."""
                raw = psb.tile([128, 512], BF16, tag="rraw")
                nc.scalar.copy(raw[:], raw_ps[:])
                swp = psb.tile([128, 512], BF16, tag="rswp")
                # partition-half swap via SBUF->SBUF DMA; triggered on the
                # otherwise-idle Sync sequencer so it never queues behind
                # the 1MB xt loads on gpsimd.
                nc.sync.dma_start(swp[0:64, :], raw[64:128, :])
                nc.sync.dma_start(swp[64:128, :], raw[0:64, :])
                t1 = psb.tile([128, 512], BF16, tag="rt1")
                nc.vector.tensor_mul(t1[:], raw[:], cos_sb[:, pos_sl])
                t2 = psb.tile([128, 512], BF16, tag="rt2")
                nc.vector.tensor_mul(t2[:], swp[:], sin_sb[:, pos_sl])
                nc.vector.tensor_add(dst, t1[:], t2[:])

            for si in range(T // 512):
                b, sl = divmod(si, NSL)
                pos_sl = bass.ts(sl, 512)
                xt_sl = xt_tiles[si]
                if si >= 2:
                    # two 1MB chunks per slice (each splits across 16 SDMA
                    # engines internally)
                    for oc in range(2):
                        osl = slice(8 * oc, 8 * oc + 8)
                        (nc.gpsimd if oc == 0 else nc.scalar).dma_start(
                            xt_sl[:, osl, :], xt[:, si, osl, :]
                        )
                if si == 3:
                    # phase-C tables (needed only at ~190us) ride behind
                    # slice 3's chunks so they don't steal startup HBM bw
                    # from wq/xt1
                    for c in range(HL):
                        nc.gpsimd.dma_start(wo_sb[:, c, :], wo[:, c, :])
                    nc.gpsimd.dma_start(msk_sb[:], msk)
                    nc.gpsimd.dma_start(ident_sb[:], ident)
                def do_q():
                    for h in range(HL):
                        q_ps = pps.tile([128, 512], F32, tag="q", name="q_ps",
                                        bufs=q_bufs)
                        for o in range(KD):
                            nc.tensor.matmul(
                                q_ps[:], wq_sb[:, o, bass.ts(h, DH)],
                                xt_sl[:, o, :],
                                start=(o == 0), stop=(o == KD - 1),
                            )
                        rope(qT_sb[:, h, b, pos_sl], q_ps, pos_sl)

                def do_kv():
                    k_ps = pps.tile([128, 512], F32, tag="k", name="k_ps",
                                    bufs=k_bufs)
                    for o in range(KD):
                        nc.tensor.matmul(
                            k_ps[:], wk_sb[:, o, :], xt_sl[:, o, :],
                            start=(o == 0), stop=(o == KD - 1),
                        )
                    rope(kT_sb[:, b, pos_sl], k_ps, pos_sl)
                    for jt in range(4):
                        v_ps = pps.tile([128, DH], F32, tag="v", name="v_ps",
                                        bufs=v_bufs)
                        for o in range(KD):
                            nc.tensor.matmul(
                                v_ps[:], xt_sl[:, o, bass.ts(jt, 128)],
                                wv_sb[:, o, :],
                                start=(o == 0), stop=(o == KD - 1),
                            )
                        nc.scalar.copy(v_sb[:, b, 4 * sl + jt, :], v_ps[:])

                if si == T // 512 - 1:
                    # last slice: q first so the phase tail is the short
                    # v-copy chain (ACT only) instead of the q-RoPE chain
                    # (ACT copy -> swap DMA -> DVE muls, ~4us), which
                    # otherwise stalls the first attention matmuls on the
                    # PSUM-bank handoff.
                    do_q()
                    do_kv()
                else:
                    do_kv()
                    do_q()

        # ---------------- phase C: attention + wo ----------------
        with tc.tile_pool(name="att_sb", bufs=asb_bufs) as asb, \
             tc.tile_pool(name="att_sb2", bufs=2) as asb2, \
             tc.tile_pool(name="att_ps", bufs=2, space="PSUM") as aps:

            def emit_wo_block(b, qs, nt, aoT, final):
                # wo + store for one 128-token row block; one [128, 4, 512]
                # f32 SBUF tile written as a single 1MB DMA (final blocks:
                # per-od 256KB DMAs so the drain starts per-copy).
                o_blk = asb.tile([128, 4, 512], F32, tag="out",
                                 bufs=out_bufs)
                row0 = b * S + qs * 512 + nt * 128
                for od in range(4):
                    w_ps = aps.tile([128, 512], F32, tag="wo", bufs=wo_bufs)
                    for c in range(HL):
                        nc.tensor.matmul(
                            w_ps[:], aoT[:, c, bass.ts(nt, 128)],
                            wo_sb[:, c, bass.ts(od, 512)],
                            start=(c == 0), stop=(c == HL - 1),
                        )
                    use_act = (
                        wo_copy == "act"
                        or (wo_copy == "mix" and (nt + od) % 2 == 0)
                    )
                    if use_act:
                        nc.scalar.copy(o_blk[:, od, :], w_ps[:])
                    else:
                        nc.vector.tensor_copy(o_blk[:, od, :], w_ps[:])
                    if final:
                        nc.sync.dma_start(
                            out[bass.ds(row0, 128), od, :], o_blk[:, od, :]
                        )
                if not final:
                    nc.sync.dma_start(out[bass.ds(row0, 128), :, :], o_blk[:])

            # wo blocks for iteration i are emitted one-per-head during
            # iteration i+1's attention loop, so the PE FIFO interleaves wo
            # matmuls with attention matmuls and each fills the other's
            # stall windows (exp latency vs psum-copy latency).
            pending_wo = []
            for b in range(BL):
                for qs in range(NSL):
                    q_sl = bass.ts(qs, 512)
                    nk = 4 * qs + 4
                    aoT = asb2.tile([128, HL, 512], BF16, tag="aoT")
                    for h in range(HL):
                        oT_ps = aps.tile([128, 512], F32, tag="oT", bufs=oT_bufs)
                        # bufs=1: keeps sc3+oT2+l1+wo2 = 8 PSUM banks
                        l_ps = aps.tile(
                            [128, 512], F32, tag="l", name="l_ps", bufs=1
                        )
                        l_started = False
                        e_quad = []
                        e_diag = []
                        for j in range(nk):
                            r = j - 4 * qs
                            masked = r >= 0
                            # columns q_local < 128 r are fully masked for
                            # the r-th diagonal tile: skip them everywhere
                            # (scores, mask, exp, PV, sums).
                            qlo = 128 * r if masked else 0
                            qg = bass.ds(qs * 512 + qlo, 512 - qlo)
                            s_ps = aps.tile([128, 512], F32, tag="sc", bufs=sc_bufs)
                            nc.tensor.matmul(
                                s_ps[:, qlo:], kT_sb[:, b, bass.ts(j, 128)],
                                qT_sb[:, h, b, qg], start=True,
                                stop=not masked,
                            )
                            if masked:
                                # scores += -30000 on causally-invalid
                                # entries; only the 128-col diagonal block
                                # can contain them, so narrow the matmul.
                                nc.tensor.matmul(
                                    s_ps[:, qlo:qlo + 128], ident_sb[:],
                                    msk_sb[:, r, qlo:qlo + 128],
                                    start=False, stop=True,
                                )
                            e_sb = asb.tile([128, 512], BF16, tag="exp", bufs=exp_bufs)
                            nc.scalar.activation(
                                e_sb[:, qlo:], s_ps[:, qlo:],
                                mybir.ActivationFunctionType.Exp, scale=scale,
                            )
                            nc.tensor.matmul(
                                oT_ps[:, qlo:], v_sb[:, b, j, :], e_sb[:, qlo:],
                                start=(j == 0), stop=(j == nk - 1),
                                skip_group_check=True,
                            )
                            if masked and l_quad:
                                # collect the 4 diagonal exp tiles; their l
                                # contribution is emitted range-batched
                                # after the j loop (see below).
                                e_diag.append(e_sb)
                            elif masked or not l_quad:
                                nc.tensor.matmul(
                                    l_ps[:, qlo:], ones_sb[:], e_sb[:, qlo:],
                                    start=not l_started, stop=(j == nk - 1),
                                    skip_group_check=True,
                                )
                                l_started = True
                            else:
                                # off-diagonal (full 512 cols): batch 4-8
                                # exp tiles into one ones-matmul via a DVE
                                # pairwise tree-sum (bf16, 2 elem/cyc/lane).
                                # group: 8 when 8+ off-diag tiles remain,
                                # else 4 (off-diag count is 4*qs).
                                e_quad.append(e_sb)
                                left = 4 * qs - (j + 1)
                                if (len(e_quad) == 8
                                        or (len(e_quad) == 4 and left < 4)):
                                    cur = e_quad
                                    while len(cur) > 1:
                                        nxt = []
                                        for a2, b2 in zip(cur[::2], cur[1::2]):
                                            t = asb.tile([128, 512], BF16,
                                                         tag="lt", bufs=8)
                                            nc.vector.tensor_add(
                                                t[:], a2[:], b2[:])
                                            nxt.append(t)
                                        cur = nxt
                                    nc.tensor.matmul(
                                        l_ps[:], ones_sb[:], cur[0][:],
                                        start=not l_started, stop=False,
                                        skip_group_check=True,
                                    )
                                    l_started = True
                                    e_quad = []
                        if e_diag:
                            # diagonal l, range-batched: the r-th diagonal
                            # tile only contributes for q >= 128r, so
                            #   l[0:128]   = e0            (overwrite range)
                            #   l[128:512] += e0+e1        (DVE pre-sum)
                            #   l[256:384] += e2
                            #   l[384:512] += e2+e3        (DVE pre-sum)
                            # 768 PE cycles instead of 1280; per-element
                            # has_written bits make the mixed overwrite/
                            # accumulate ranges compose.
                            e0, e1, e2, e3 = e_diag
                            da = asb.tile([128, 512], BF16, tag="lda", bufs=2)
                            nc.vector.tensor_add(
                                da[:, 128:], e0[:, 128:], e1[:, 128:])
                            db = asb.tile([128, 512], BF16, tag="ldb", bufs=2)
                            nc.vector.tensor_add(
                                db[:, 384:], e2[:, 384:], e3[:, 384:])
                            nc.tensor.matmul(
                                l_ps[:, 0:128], ones_sb[:], e0[:, 0:128],
                                start=not l_started, stop=False,
                                skip_group_check=True,
                            )
                            nc.tensor.matmul(
                                l_ps[:, 128:], ones_sb[:], da[:, 128:],
                                start=False, stop=False,
                                skip_group_check=True,
                            )
                            nc.tensor.matmul(
                                l_ps[:, 256:384], ones_sb[:],
                                e2[:, 256:384],
                                start=False, stop=False,
                                skip_group_check=True,
                            )
                            nc.tensor.matmul(
                                l_ps[:, 384:], ones_sb[:], db[:, 384:],
                                start=False, stop=True,
                                skip_group_check=True,
                            )
                        rc_sb = asb.tile([128, 512], F32, tag="rc")
                        nc.vector.reciprocal_approx_fast(rc_sb[:], l_ps[:])
                        nc.vector.tensor_mul(aoT[:, h, :], oT_ps[:], rc_sb[:])
                        if pending_wo:
                            pending_wo.pop(0)()
                    final = (b == BL - 1 and qs == NSL - 1)
                    for nt in range(4):
                        pending_wo.append(
                            (lambda b=b, qs=qs, nt=nt, aoT=aoT, f=final:
                             emit_wo_block(b, qs, nt, aoT, f))
                        )
            for go in pending_wo:
                go()
    nc.finalize()
    return nc


_NC_CACHE = {}


def _get_nc():
    if "nc" not in _NC_CACHE:
        _NC_CACHE["nc"] = build_nc()
    return _NC_CACHE["nc"]


def kernel(x, freqs_cos, freqs_sin, wq, wk, wv, wo):
    x = np.asarray(x)
    freqs_cos = np.asarray(freqs_cos)
    freqs_sin = np.asarray(freqs_sin)
    wq = np.asarray(wq)
    wk = np.asarray(wk)
    wv = np.asarray(wv)
    wo = np.asarray(wo)
    bf = ml_dtypes.bfloat16
    perm = np.concatenate([np.arange(0, DH, 2), np.arange(1, DH, 2)])

    wq_p = wq.reshape(D, NH, DH)[:, :, perm].reshape(D, NH * DH)
    wk_p = wk.reshape(D, NKV, DH)[:, :, perm].reshape(D, NKV * DH)

    cosT = freqs_cos.T  # [64, S]
    sinT = freqs_sin.T
    c2 = np.ascontiguousarray(np.concatenate([cosT, cosT], axis=0)).astype(bf)
    s2 = np.ascontiguousarray(np.concatenate([-sinT, sinT], axis=0)).astype(bf)

    # additive mask[k, r, q] = 0 if causally valid (q - k - 128 r >= 0),
    # else -30000 (exp underflows to exactly 0 after 1/sqrt(dh) scaling)
    kk = np.arange(128)[:, None, None]
    rr = np.arange(4)[None, :, None]
    qq = np.arange(512)[None, None, :]
    masks = np.ascontiguousarray(
        np.where(qq - kk - 128 * rr >= 0, 0.0, -30000.0)
    ).astype(bf)
    identity = np.eye(128, dtype=np.float32).astype(bf)

    def tile_weight(w, dt):
        # [D, C] -> [128, KD, C] with w[o*128+p, c] -> [p, o, c], contiguous
        c = w.shape[1]
        return np.ascontiguousarray(
            w.reshape(KD, 128, c).transpose(1, 0, 2)
        ).astype(dt)

    in_maps = []
    for core in range(8):
        dp, tp = divmod(core, TP)
        xs = x[dp * BL: (dp + 1) * BL].reshape(T, D)
        # [D, T] -> [128, T//512, KD, 512]: x.T[o*128+p, sl*512+t] ->
        # [p, sl, o, t], fully contiguous per partition line
        xt = np.ascontiguousarray(
            xs.T.reshape(KD, 128, T // 512, 512).transpose(1, 2, 0, 3)
        ).astype(bf)
        wo_sl = wo[tp * QC: (tp + 1) * QC, :]
        wo_t = np.ascontiguousarray(
            wo_sl.reshape(HL, 128, D).transpose(1, 0, 2)
        ).astype(bf)
        in_maps.append(
            {
                "xt": xt,
                "wq": tile_weight(wq_p[:, tp * QC: (tp + 1) * QC], bf),
                "wk": tile_weight(wk_p[:, tp * DH: (tp + 1) * DH], bf),
                "wv": tile_weight(wv[:, tp * DH: (tp + 1) * DH], bf),
                "wo": wo_t,
                "cos2": c2,
                "sin2": s2,
                "msk": masks,
                "ident": identity,
            }
        )

    nc = _get_nc()
    res = run_bass_kernel_spmd(nc, in_maps, core_ids=list(range(8)))
    _NC_CACHE["last_results"] = res

    full = np.zeros((B, S, D), dtype=np.float32)
    for core in range(8):
        dp = core // TP
        full[dp * BL: (dp + 1) * BL] += (
            res.results[core]["out"].astype(np.float32).reshape(BL, S, D)
        )
    return full
